# revision 20
# baseline (speedup 1.0000x reference)
"""Trainium2 Bass kernel for NeighborAggregator (gather -> segment_sum -> softmax).

Strategy (8 NeuronCores, SPMD):
  - Row-shard the N=16384 instances: each core owns R=2048 rows = 65536 edges.
  - Host converts input_tensor to bf16, halving the streamed bytes: each core
    streams its 64MiB row shard through SBUF in 16 blocks of 128 rows
    (32KiB/partition); gathers d[p,j] = row_p[idx[p,j]] on-chip with one
    gpsimd ap_gather per block (bf16 row fits the 32KB/partition limit in a
    single call) + diagonal-mask select + strided reduce.
  - contrib = d * values (bf16 for the PE masks).
  - Segment-sum over C=16384 columns via PE outer-product histogram in bf16:
    tile t = one edge per partition,
      a_m[p, h] = contrib_p * [hi_p == h]   (hi = idx >> 7)
      b_eq[p, l] = [lo_p == l]              (lo = idx & 127)
    a-side built on DVE with batched broadcast-iota tensor_tensor ops
    (TB=8 tiles per instruction); b-side split between DVE (batched TT) and
    the scalar engine (Relu(1 - (iota - lo)^2) = one-hot, Square+Relu with
    per-partition bias) to balance engine load. PE accumulates
    psum[h,l] += a_m^T @ b_eq over all 512 tiles at bf16 rate.
  - AllReduce the [128,128] f32 partials across the 8 cores, then softmax
    on-device; every core writes the identical full outputs.
"""

import sys

for _p in ("/opt/trn_rl_repo", "/root/.axon_site/_ro/trn_rl_repo"):
    if _p not in sys.path:
        sys.path.append(_p)

import numpy as np

N = 16384
C = 16384
K = 32
NCORES = 8
P = 128

R = N // NCORES          # rows per core
NBLK = R // P            # row blocks per core (16)
EPP = NBLK * K           # edges per partition (512) == number of tiles
TB = 8                   # tiles per batched mask group
NGRP = EPP // TB         # mask groups (64)

_CACHE = {}


def _build_program():
    from concourse import bacc, bass_isa, mybir, tile

    f32 = mybir.dt.float32
    bf16 = mybir.dt.bfloat16
    fp16 = mybir.dt.float16
    i32 = mybir.dt.int32
    i16 = mybir.dt.int16

    nc = bacc.Bacc("TRN2", target_bir_lowering=False, debug=False,
                   num_devices=NCORES)

    rows_d = nc.dram_tensor("rows", [R, C], fp16, kind="ExternalInput")
    idx_d = nc.dram_tensor("idx", [R, K], i32, kind="ExternalInput")
    vals_d = nc.dram_tensor("vals", [R, K], f32, kind="ExternalInput")
    alpha_d = nc.dram_tensor("alpha", [P, P], f32, kind="ExternalOutput")
    red_d = nc.dram_tensor("red", [P, P], f32, kind="ExternalOutput")

    with tile.TileContext(nc) as tc:
        with tc.tile_pool(name="fix", bufs=1) as fix, \
             tc.tile_pool(name="rows", bufs=3) as rows_pool, \
             tc.tile_pool(name="work", bufs=3) as work, \
             tc.tile_pool(name="maeq", bufs=3) as maeq_pool, \
             tc.tile_pool(name="mam", bufs=4) as mam_pool, \
             tc.tile_pool(name="mb", bufs=3) as mb_pool, \
             tc.tile_pool(name="mbact", bufs=20) as mbact_pool, \
             tc.tile_pool(name="psum", bufs=1, space="PSUM") as psum_pool, \
             tc.tile_pool(name="dram", bufs=1, space="DRAM") as dram_pool:

            # ---- one-time constants ----
            iota_i = fix.tile([P, P], i32)
            nc.gpsimd.iota(iota_i[:], pattern=[[1, P]], base=0,
                           channel_multiplier=0)
            iota_b = fix.tile([P, P], f32)
            nc.vector.tensor_copy(out=iota_b[:], in_=iota_i[:])

            # diagonal mask for ap_gather extraction:
            # dmask[p, s*16 + k] = (k == p % 16)
            fmod = fix.tile([P, K * 16], i32)
            nc.gpsimd.iota(fmod[:], pattern=[[0, K], [1, 16]], base=0,
                           channel_multiplier=0)
            pidx = fix.tile([P, 1], i32)
            nc.gpsimd.iota(pidx[:], pattern=[[0, 1]], base=0,
                           channel_multiplier=1)
            pmod = fix.tile([P, 1], i32)
            nc.vector.tensor_scalar(out=pmod[:], in0=pidx[:], scalar1=15,
                                    scalar2=None,
                                    op0=mybir.AluOpType.bitwise_and)
            dmask = fix.tile([P, K * 16], fp16)
            nc.vector.tensor_tensor(out=dmask[:], in0=fmod[:],
                                    in1=pmod[:].to_broadcast([P, K * 16]),
                                    op=mybir.AluOpType.is_equal)

            # ---- load idx/vals: [R, K] -> [P, NBLK*K] ----
            idx_sb = fix.tile([P, EPP], i32)
            nc.sync.dma_start(
                out=idx_sb[:].rearrange("p (b k) -> p b k", k=K),
                in_=idx_d[:].rearrange("(b p) k -> p b k", p=P))
            vals_sb = fix.tile([P, EPP], f32)
            nc.sync.dma_start(
                out=vals_sb[:].rearrange("p (b k) -> p b k", k=K),
                in_=vals_d[:].rearrange("(b p) k -> p b k", p=P))

            # ---- idx-derived streams ----
            # pair index for the f32-pair gather from bf16 rows
            idxp_i = fix.tile([P, EPP], i32)
            nc.vector.tensor_scalar(out=idxp_i[:], in0=idx_sb[:], scalar1=1,
                                    scalar2=None,
                                    op0=mybir.AluOpType.logical_shift_right)
            idx16 = fix.tile([P, EPP], i16)
            nc.vector.tensor_copy(out=idx16[:], in_=idxp_i[:])
            # half-bit hb = idx & 1, split vals into v0/v1 = vals*(1-hb), vals*hb
            hb_i = fix.tile([P, EPP], i32)
            nc.vector.tensor_scalar(out=hb_i[:], in0=idx_sb[:], scalar1=1,
                                    scalar2=None,
                                    op0=mybir.AluOpType.bitwise_and)
            hbf = fix.tile([P, EPP], f32)
            nc.vector.tensor_copy(out=hbf[:], in_=hb_i[:])
            v1 = fix.tile([P, EPP], f32)
            nc.vector.tensor_tensor(out=v1[:], in0=vals_sb[:], in1=hbf[:],
                                    op=mybir.AluOpType.mult)
            v0 = fix.tile([P, EPP], f32)
            nc.vector.tensor_tensor(out=v0[:], in0=vals_sb[:], in1=v1[:],
                                    op=mybir.AluOpType.subtract)
            hi_i = fix.tile([P, EPP], i32)
            nc.vector.tensor_scalar(out=hi_i[:], in0=idx_sb[:], scalar1=7,
                                    scalar2=None,
                                    op0=mybir.AluOpType.logical_shift_right)
            hif = fix.tile([P, EPP], f32)
            nc.vector.tensor_copy(out=hif[:], in_=hi_i[:])
            lo_i = fix.tile([P, EPP], i32)
            nc.vector.tensor_scalar(out=lo_i[:], in0=idx_sb[:], scalar1=127,
                                    scalar2=None,
                                    op0=mybir.AluOpType.bitwise_and)
            lof = fix.tile([P, EPP], f32)
            nc.vector.tensor_copy(out=lof[:], in_=lo_i[:])
            # negated f32 lo for the scalar-engine one-hot bias
            nlof = fix.tile([P, EPP], f32)
            nc.vector.tensor_scalar(out=nlof[:], in0=lo_i[:], scalar1=-1,
                                    scalar2=None, op0=mybir.AluOpType.mult)

            cf32 = fix.tile([P, EPP], f32)

            # two independent accumulators: even groups (DVE-built masks)
            # go to psumA inline; odd groups (scalar-engine lo one-hots)
            # are deferred to the end of each block into psumB, so the
            # in-order PE queue never blocks mid-block on the slower
            # scalar engine.
            psumA = psum_pool.tile([P, P], f32)
            psumB = psum_pool.tile([P, P], f32)
            n_tiles_per_bank = EPP // 2
            mm_count = {"A": 0, "B": 0}
            pending_b = []

            def mm(bank, psum_t, lhsT, rhs):
                i = mm_count[bank]
                nc.tensor.matmul(out=psum_t[:], lhsT=lhsT, rhs=rhs,
                                 start=(i == 0),
                                 stop=(i == n_tiles_per_bank - 1),
                                 skip_group_check=True)
                mm_count[bank] = i + 1

            def flush_pending():
                for lhsT, rhs in pending_b:
                    mm("B", psumB, lhsT, rhs)
                pending_b.clear()

            def emit_select(rb):
                # gather+select for block rb: d values -> cf32[:, block slice]
                r0 = rb * P
                ks = slice(rb * K, (rb + 1) * K)
                rows_blk = rows_pool.tile([P, C // 2], f32, tag="rows")
                nc.sync.dma_start(out=rows_blk[:],
                                  in_=rows_d[r0:r0 + P, :].bitcast(f32))

                # on-chip gather of the f32 pair holding each target fp16
                # (idx i applied to all 16 partitions of each Q7 group):
                # gath[p, s*16+k] = rows_blk[p, idxp[(g*16)+k, s]]
                gath = work.tile([P, K * 16], f32, tag="gath")
                nc.gpsimd.ap_gather(out_ap=gath[:], in_ap=rows_blk[:],
                                    idxs_ap=idx16[:, ks],
                                    channels=P, num_elems=C // 2, d=1,
                                    num_idxs=K * 16)
                gb = gath[:].bitcast(fp16).rearrange("p (x h) -> p x h", h=2)
                sel0 = work.tile([P, K * 16], fp16, tag="sel0")
                nc.vector.tensor_tensor(out=sel0[:], in0=gb[:, :, 0],
                                        in1=dmask[:],
                                        op=mybir.AluOpType.mult)
                sel1 = work.tile([P, K * 16], fp16, tag="sel1")
                nc.vector.tensor_tensor(out=sel1[:], in0=gb[:, :, 1],
                                        in1=dmask[:],
                                        op=mybir.AluOpType.mult)
                dv0 = work.tile([P, K], f32, tag="dv0")
                nc.vector.tensor_reduce(
                    out=dv0[:],
                    in_=sel0[:].rearrange("p (s k) -> p s k", k=16),
                    axis=mybir.AxisListType.X,
                    op=mybir.AluOpType.add)
                dv1 = work.tile([P, K], f32, tag="dv1")
                nc.vector.tensor_reduce(
                    out=dv1[:],
                    in_=sel1[:].rearrange("p (s k) -> p s k", k=16),
                    axis=mybir.AxisListType.X,
                    op=mybir.AluOpType.add)
                # contrib = dv0*v0 + dv1*v1 (selects the right fp16 half)
                c0t = work.tile([P, K], f32, tag="c0t")
                nc.vector.tensor_tensor(out=c0t[:], in0=dv0[:],
                                        in1=v0[:, ks],
                                        op=mybir.AluOpType.mult)
                c1t = work.tile([P, K], f32, tag="c1t")
                nc.vector.tensor_tensor(out=c1t[:], in0=dv1[:],
                                        in1=v1[:, ks],
                                        op=mybir.AluOpType.mult)
                nc.vector.tensor_tensor(out=cf32[:, ks], in0=c0t[:],
                                        in1=c1t[:], op=mybir.AluOpType.add)

            def emit_masks(rb):
                # mask build + matmuls for block rb (runs one block behind
                # the gather/select so cross-engine waits resolve early)
                for gg in range(K // TB):
                    g = rb * (K // TB) + gg
                    t0 = g * TB

                    hi_bc = hif[:, t0:t0 + TB][:, :, None].broadcast_to(
                        [P, TB, P])
                    io_bc = iota_b[:][:, None, :].broadcast_to([P, TB, P])
                    c_bc = cf32[:, t0:t0 + TB][:, :, None].broadcast_to(
                        [P, TB, P])

                    a_eq = maeq_pool.tile([P, TB, P], f32, tag="a_eq")
                    nc.vector.tensor_tensor(out=a_eq[:], in0=hi_bc,
                                            in1=io_bc,
                                            op=mybir.AluOpType.is_equal)
                    a_m = mam_pool.tile([P, TB, P], f32, tag="a_m")
                    nc.vector.tensor_tensor(out=a_m[:], in0=a_eq[:],
                                            in1=c_bc,
                                            op=mybir.AluOpType.mult)

                    if g % 2 == 1:
                        # scalar engine builds the lo one-hot via
                        # Relu(1 - (iota - lo)^2); matmuls deferred
                        for t in range(TB):
                            T = t0 + t
                            sq = mbact_pool.tile([P, P], f32, tag="sq")
                            nc.scalar.activation(
                                out=sq[:], in_=iota_b[:],
                                func=mybir.ActivationFunctionType.Square,
                                bias=nlof[:, T:T + 1], scale=1.0)
                            bt = mbact_pool.tile([P, P], f32, tag="bt")
                            nc.scalar.activation(
                                out=bt[:], in_=sq[:],
                                func=mybir.ActivationFunctionType.Relu,
                                bias=1.0, scale=-1.0)
                            pending_b.append((a_m[:, t, :], bt[:]))
                    else:
                        lo_bc = lof[:, t0:t0 + TB][:, :, None].broadcast_to(
                            [P, TB, P])
                        b_eq = mb_pool.tile([P, TB, P], f32, tag="b_eq")
                        nc.vector.tensor_tensor(out=b_eq[:], in0=lo_bc,
                                                in1=io_bc,
                                                op=mybir.AluOpType.is_equal)
                        for t in range(TB):
                            mm("A", psumA, a_m[:, t, :], b_eq[:, t, :])

                flush_pending()

            for rb in range(NBLK):
                if rb > 0:
                    emit_masks(rb - 1)
                emit_select(rb)
            emit_masks(NBLK - 1)

            assert mm_count["A"] == mm_count["B"] == n_tiles_per_bank

            red_tmp = fix.tile([P, P], f32)
            nc.vector.tensor_copy(out=red_tmp[:], in_=psumA[:])
            red_sb = fix.tile([P, P], f32)
            nc.vector.tensor_tensor(out=red_sb[:], in0=red_tmp[:],
                                    in1=psumB[:], op=mybir.AluOpType.add)

            partial = dram_pool.tile([P, P], f32)
            nc.sync.dma_start(out=partial[:], in_=red_sb[:])
            allred = dram_pool.tile([P, P], f32)
            nc.gpsimd.collective_compute(
                "AllReduce",
                mybir.AluOpType.add,
                replica_groups=[list(range(NCORES))],
                ins=[partial[:].opt()],
                outs=[allred[:].opt()],
            )
            r_sb = fix.tile([P, P], f32)
            nc.sync.dma_start(out=r_sb[:], in_=allred[:])
            nc.sync.dma_start(out=red_d[:], in_=r_sb[:])

            # softmax over all 16384 entries of r_sb
            pm = fix.tile([P, 1], f32)
            nc.vector.tensor_reduce(out=pm[:], in_=r_sb[:],
                                    axis=mybir.AxisListType.X,
                                    op=mybir.AluOpType.max)
            gm = fix.tile([P, 1], f32)
            nc.gpsimd.partition_all_reduce(gm[:], pm[:], channels=P,
                                           reduce_op=bass_isa.ReduceOp.max)
            negm = fix.tile([P, 1], f32)
            nc.vector.tensor_scalar_mul(negm[:], gm[:], -1.0)
            e_sb = fix.tile([P, P], f32)
            s_sb = fix.tile([P, 1], f32)
            nc.scalar.activation(out=e_sb[:], in_=r_sb[:],
                                 func=mybir.ActivationFunctionType.Exp,
                                 bias=negm[:], scale=1.0, accum_out=s_sb[:])
            stot = fix.tile([P, 1], f32)
            nc.gpsimd.partition_all_reduce(stot[:], s_sb[:], channels=P,
                                           reduce_op=bass_isa.ReduceOp.add)
            rec = fix.tile([P, 1], f32)
            nc.vector.reciprocal(rec[:], stot[:])
            alpha_sb = fix.tile([P, P], f32)
            nc.scalar.activation(out=alpha_sb[:], in_=e_sb[:],
                                 func=mybir.ActivationFunctionType.Copy,
                                 scale=rec[:])
            nc.sync.dma_start(out=alpha_d[:], in_=alpha_sb[:])

    nc.compile()
    return nc


def _get_program():
    if "nc" not in _CACHE:
        _CACHE["nc"] = _build_program()
    return _CACHE["nc"]


def make_in_maps(input_tensor, indices, values):
    input_tensor = np.asarray(input_tensor)
    indices = np.asarray(indices)
    values = np.ascontiguousarray(np.asarray(values, dtype=np.float32))
    in_maps = []
    for m in range(NCORES):
        r0, r1 = m * R, (m + 1) * R
        in_maps.append({
            "rows": np.ascontiguousarray(
                input_tensor[r0:r1].astype(np.float16)),
            "idx": np.ascontiguousarray(indices[r0:r1].astype(np.int32)),
            "vals": values[r0:r1],
        })
    return in_maps


def kernel(input_tensor, indices, values, k=K, **_unused):
    assert int(k) == K
    from concourse.bass_utils import run_bass_kernel_spmd

    nc = _get_program()
    in_maps = make_in_maps(input_tensor, indices, values)
    res = run_bass_kernel_spmd(nc, in_maps, list(range(NCORES)))
    out0 = res.results[0]
    alpha = np.asarray(out0["alpha"], dtype=np.float32).reshape(C)
    reduced = np.asarray(out0["red"], dtype=np.float32).reshape(C)
    return alpha, reduced


# revision 21
# speedup vs baseline: 1.1017x; 1.1017x over previous
"""Trainium2 Bass kernel for NeighborAggregator (gather -> segment_sum -> softmax).

Strategy (8 NeuronCores, SPMD):
  - Row-shard the N=16384 instances: each core owns R=2048 rows = 65536 edges.
  - Host converts input_tensor to bf16, halving the streamed bytes: each core
    streams its 64MiB row shard through SBUF in 16 blocks of 128 rows
    (32KiB/partition); gathers d[p,j] = row_p[idx[p,j]] on-chip with one
    gpsimd ap_gather per block (bf16 row fits the 32KB/partition limit in a
    single call) + diagonal-mask select + strided reduce.
  - contrib = d * values (bf16 for the PE masks).
  - Segment-sum over C=16384 columns via PE outer-product histogram in bf16:
    tile t = one edge per partition,
      a_m[p, h] = contrib_p * [hi_p == h]   (hi = idx >> 7)
      b_eq[p, l] = [lo_p == l]              (lo = idx & 127)
    a-side built on DVE with batched broadcast-iota tensor_tensor ops
    (TB=8 tiles per instruction); b-side split between DVE (batched TT) and
    the scalar engine (Relu(1 - (iota - lo)^2) = one-hot, Square+Relu with
    per-partition bias) to balance engine load. PE accumulates
    psum[h,l] += a_m^T @ b_eq over all 512 tiles at bf16 rate.
  - AllReduce the [128,128] f32 partials across the 8 cores, then softmax
    on-device; every core writes the identical full outputs.
"""

import sys

for _p in ("/opt/trn_rl_repo", "/root/.axon_site/_ro/trn_rl_repo"):
    if _p not in sys.path:
        sys.path.append(_p)

import numpy as np

N = 16384
C = 16384
K = 32
NCORES = 8
P = 128

R = N // NCORES          # rows per core
NBLK = R // P            # row blocks per core (16)
EPP = NBLK * K           # edges per partition (512) == number of tiles
TB = 8                   # tiles per batched mask group
NGRP = EPP // TB         # mask groups (64)

_CACHE = {}


def _build_program():
    from concourse import bacc, bass_isa, mybir, tile

    f32 = mybir.dt.float32
    bf16 = mybir.dt.bfloat16
    fp16 = mybir.dt.float16
    i32 = mybir.dt.int32
    i16 = mybir.dt.int16

    nc = bacc.Bacc("TRN2", target_bir_lowering=False, debug=False,
                   num_devices=NCORES)

    rows_d = nc.dram_tensor("rows", [R, C], fp16, kind="ExternalInput")
    idx_d = nc.dram_tensor("idx", [R, K], i32, kind="ExternalInput")
    vals_d = nc.dram_tensor("vals", [R, K], f32, kind="ExternalInput")
    alpha_d = nc.dram_tensor("alpha", [P, P], f32, kind="ExternalOutput")
    red_d = nc.dram_tensor("red", [P, P], f32, kind="ExternalOutput")

    with tile.TileContext(nc) as tc:
        with tc.tile_pool(name="fix", bufs=1) as fix, \
             tc.tile_pool(name="rows", bufs=3) as rows_pool, \
             tc.tile_pool(name="work", bufs=3) as work, \
             tc.tile_pool(name="maeq", bufs=3) as maeq_pool, \
             tc.tile_pool(name="mam", bufs=4) as mam_pool, \
             tc.tile_pool(name="mb", bufs=3) as mb_pool, \
             tc.tile_pool(name="mbact", bufs=20) as mbact_pool, \
             tc.tile_pool(name="psum", bufs=1, space="PSUM") as psum_pool, \
             tc.tile_pool(name="dram", bufs=1, space="DRAM") as dram_pool:

            # ---- one-time constants ----
            iota_i = fix.tile([P, P], i32)
            nc.gpsimd.iota(iota_i[:], pattern=[[1, P]], base=0,
                           channel_multiplier=0)
            iota_b = fix.tile([P, P], f32)
            nc.vector.tensor_copy(out=iota_b[:], in_=iota_i[:])

            # diagonal mask for ap_gather extraction:
            # dmask[p, s*16 + k] = (k == p % 16)
            fmod = fix.tile([P, K * 16], i32)
            nc.gpsimd.iota(fmod[:], pattern=[[0, K], [1, 16]], base=0,
                           channel_multiplier=0)
            pidx = fix.tile([P, 1], i32)
            nc.gpsimd.iota(pidx[:], pattern=[[0, 1]], base=0,
                           channel_multiplier=1)
            pmod = fix.tile([P, 1], i32)
            nc.vector.tensor_scalar(out=pmod[:], in0=pidx[:], scalar1=15,
                                    scalar2=None,
                                    op0=mybir.AluOpType.bitwise_and)
            dmask = fix.tile([P, K * 16], fp16)
            nc.vector.tensor_tensor(out=dmask[:], in0=fmod[:],
                                    in1=pmod[:].to_broadcast([P, K * 16]),
                                    op=mybir.AluOpType.is_equal)

            # ---- load idx/vals: [R, K] -> [P, NBLK*K] ----
            idx_sb = fix.tile([P, EPP], i32)
            nc.sync.dma_start(
                out=idx_sb[:].rearrange("p (b k) -> p b k", k=K),
                in_=idx_d[:].rearrange("(b p) k -> p b k", p=P))
            vals_sb = fix.tile([P, EPP], f32)
            nc.sync.dma_start(
                out=vals_sb[:].rearrange("p (b k) -> p b k", k=K),
                in_=vals_d[:].rearrange("(b p) k -> p b k", p=P))

            # ---- idx-derived streams ----
            # pair index for the f32-pair gather from bf16 rows
            idxp_i = fix.tile([P, EPP], i32)
            nc.vector.tensor_scalar(out=idxp_i[:], in0=idx_sb[:], scalar1=1,
                                    scalar2=None,
                                    op0=mybir.AluOpType.logical_shift_right)
            idx16 = fix.tile([P, EPP], i16)
            nc.vector.tensor_copy(out=idx16[:], in_=idxp_i[:])
            # half-bit hb = idx & 1, split vals into v0/v1 = vals*(1-hb), vals*hb
            hb_i = fix.tile([P, EPP], i32)
            nc.vector.tensor_scalar(out=hb_i[:], in0=idx_sb[:], scalar1=1,
                                    scalar2=None,
                                    op0=mybir.AluOpType.bitwise_and)
            hbf = fix.tile([P, EPP], f32)
            nc.vector.tensor_copy(out=hbf[:], in_=hb_i[:])
            v1 = fix.tile([P, EPP], f32)
            nc.vector.tensor_tensor(out=v1[:], in0=vals_sb[:], in1=hbf[:],
                                    op=mybir.AluOpType.mult)
            v0 = fix.tile([P, EPP], f32)
            nc.vector.tensor_tensor(out=v0[:], in0=vals_sb[:], in1=v1[:],
                                    op=mybir.AluOpType.subtract)
            hi_i = fix.tile([P, EPP], i32)
            nc.vector.tensor_scalar(out=hi_i[:], in0=idx_sb[:], scalar1=7,
                                    scalar2=None,
                                    op0=mybir.AluOpType.logical_shift_right)
            hif = fix.tile([P, EPP], f32)
            nc.vector.tensor_copy(out=hif[:], in_=hi_i[:])
            lo_i = fix.tile([P, EPP], i32)
            nc.vector.tensor_scalar(out=lo_i[:], in0=idx_sb[:], scalar1=127,
                                    scalar2=None,
                                    op0=mybir.AluOpType.bitwise_and)
            lof = fix.tile([P, EPP], f32)
            nc.vector.tensor_copy(out=lof[:], in_=lo_i[:])
            # negated f32 lo for the scalar-engine one-hot bias
            nlof = fix.tile([P, EPP], f32)
            nc.vector.tensor_scalar(out=nlof[:], in0=lo_i[:], scalar1=-1,
                                    scalar2=None, op0=mybir.AluOpType.mult)

            cf32 = fix.tile([P, EPP], f32)

            # two independent accumulators: even groups (DVE-built masks)
            # go to psumA inline; odd groups (scalar-engine lo one-hots)
            # are deferred to the end of each block into psumB, so the
            # in-order PE queue never blocks mid-block on the slower
            # scalar engine.
            psumA = psum_pool.tile([P, P], f32)
            psumB = psum_pool.tile([P, P], f32)
            n_tiles_per_bank = EPP // 2
            mm_count = {"A": 0, "B": 0}
            pending_b = []

            def mm(bank, psum_t, lhsT, rhs):
                i = mm_count[bank]
                nc.tensor.matmul(out=psum_t[:], lhsT=lhsT, rhs=rhs,
                                 start=(i == 0),
                                 stop=(i == n_tiles_per_bank - 1),
                                 skip_group_check=True)
                mm_count[bank] = i + 1

            def flush_pending():
                for lhsT, rhs in pending_b:
                    mm("B", psumB, lhsT, rhs)
                pending_b.clear()

            def emit_select(rb):
                # gather+select for block rb: d values -> cf32[:, block slice]
                r0 = rb * P
                ks = slice(rb * K, (rb + 1) * K)
                rows_blk = rows_pool.tile([P, C // 2], f32, tag="rows")
                nc.gpsimd.dma_start(out=rows_blk[:],
                                    in_=rows_d[r0:r0 + P, :].bitcast(f32))

                # on-chip gather of the f32 pair holding each target fp16
                # (idx i applied to all 16 partitions of each Q7 group):
                # gath[p, s*16+k] = rows_blk[p, idxp[(g*16)+k, s]]
                gath = work.tile([P, K * 16], f32, tag="gath")
                nc.gpsimd.ap_gather(out_ap=gath[:], in_ap=rows_blk[:],
                                    idxs_ap=idx16[:, ks],
                                    channels=P, num_elems=C // 2, d=1,
                                    num_idxs=K * 16)
                gb = gath[:].bitcast(fp16).rearrange("p (x h) -> p x h", h=2)
                sel0 = work.tile([P, K * 16], fp16, tag="sel0")
                nc.vector.tensor_tensor(out=sel0[:], in0=gb[:, :, 0],
                                        in1=dmask[:],
                                        op=mybir.AluOpType.mult)
                sel1 = work.tile([P, K * 16], fp16, tag="sel1")
                nc.vector.tensor_tensor(out=sel1[:], in0=gb[:, :, 1],
                                        in1=dmask[:],
                                        op=mybir.AluOpType.mult)
                dv0 = work.tile([P, K], f32, tag="dv0")
                nc.vector.tensor_reduce(
                    out=dv0[:],
                    in_=sel0[:].rearrange("p (s k) -> p s k", k=16),
                    axis=mybir.AxisListType.X,
                    op=mybir.AluOpType.add)
                dv1 = work.tile([P, K], f32, tag="dv1")
                nc.vector.tensor_reduce(
                    out=dv1[:],
                    in_=sel1[:].rearrange("p (s k) -> p s k", k=16),
                    axis=mybir.AxisListType.X,
                    op=mybir.AluOpType.add)
                # contrib = dv0*v0 + dv1*v1 (selects the right fp16 half)
                c0t = work.tile([P, K], f32, tag="c0t")
                nc.vector.tensor_tensor(out=c0t[:], in0=dv0[:],
                                        in1=v0[:, ks],
                                        op=mybir.AluOpType.mult)
                c1t = work.tile([P, K], f32, tag="c1t")
                nc.vector.tensor_tensor(out=c1t[:], in0=dv1[:],
                                        in1=v1[:, ks],
                                        op=mybir.AluOpType.mult)
                nc.vector.tensor_tensor(out=cf32[:, ks], in0=c0t[:],
                                        in1=c1t[:], op=mybir.AluOpType.add)

            def emit_masks(rb):
                # mask build + matmuls for block rb (runs one block behind
                # the gather/select so cross-engine waits resolve early)
                for gg in range(K // TB):
                    g = rb * (K // TB) + gg
                    t0 = g * TB

                    hi_bc = hif[:, t0:t0 + TB][:, :, None].broadcast_to(
                        [P, TB, P])
                    io_bc = iota_b[:][:, None, :].broadcast_to([P, TB, P])
                    c_bc = cf32[:, t0:t0 + TB][:, :, None].broadcast_to(
                        [P, TB, P])

                    a_eq = maeq_pool.tile([P, TB, P], f32, tag="a_eq")
                    nc.vector.tensor_tensor(out=a_eq[:], in0=hi_bc,
                                            in1=io_bc,
                                            op=mybir.AluOpType.is_equal)
                    a_m = mam_pool.tile([P, TB, P], f32, tag="a_m")
                    nc.vector.tensor_tensor(out=a_m[:], in0=a_eq[:],
                                            in1=c_bc,
                                            op=mybir.AluOpType.mult)

                    if g % 2 == 1:
                        # scalar engine builds the lo one-hot via
                        # Relu(1 - (iota - lo)^2); matmuls deferred
                        for t in range(TB):
                            T = t0 + t
                            sq = mbact_pool.tile([P, P], f32, tag="sq")
                            nc.scalar.activation(
                                out=sq[:], in_=iota_b[:],
                                func=mybir.ActivationFunctionType.Square,
                                bias=nlof[:, T:T + 1], scale=1.0)
                            bt = mbact_pool.tile([P, P], f32, tag="bt")
                            nc.scalar.activation(
                                out=bt[:], in_=sq[:],
                                func=mybir.ActivationFunctionType.Relu,
                                bias=1.0, scale=-1.0)
                            pending_b.append((a_m[:, t, :], bt[:]))
                    else:
                        lo_bc = lof[:, t0:t0 + TB][:, :, None].broadcast_to(
                            [P, TB, P])
                        b_eq = mb_pool.tile([P, TB, P], f32, tag="b_eq")
                        nc.vector.tensor_tensor(out=b_eq[:], in0=lo_bc,
                                                in1=io_bc,
                                                op=mybir.AluOpType.is_equal)
                        for t in range(TB):
                            mm("A", psumA, a_m[:, t, :], b_eq[:, t, :])

                flush_pending()

            for rb in range(NBLK):
                if rb > 0:
                    emit_masks(rb - 1)
                emit_select(rb)
            emit_masks(NBLK - 1)

            assert mm_count["A"] == mm_count["B"] == n_tiles_per_bank

            red_tmp = fix.tile([P, P], f32)
            nc.vector.tensor_copy(out=red_tmp[:], in_=psumA[:])
            red_sb = fix.tile([P, P], f32)
            nc.vector.tensor_tensor(out=red_sb[:], in0=red_tmp[:],
                                    in1=psumB[:], op=mybir.AluOpType.add)

            partial = dram_pool.tile([P, P], f32)
            nc.sync.dma_start(out=partial[:], in_=red_sb[:])
            allred = dram_pool.tile([P, P], f32)
            nc.gpsimd.collective_compute(
                "AllReduce",
                mybir.AluOpType.add,
                replica_groups=[list(range(NCORES))],
                ins=[partial[:].opt()],
                outs=[allred[:].opt()],
            )
            r_sb = fix.tile([P, P], f32)
            nc.sync.dma_start(out=r_sb[:], in_=allred[:])
            nc.sync.dma_start(out=red_d[:], in_=r_sb[:])

            # softmax over all 16384 entries of r_sb
            pm = fix.tile([P, 1], f32)
            nc.vector.tensor_reduce(out=pm[:], in_=r_sb[:],
                                    axis=mybir.AxisListType.X,
                                    op=mybir.AluOpType.max)
            gm = fix.tile([P, 1], f32)
            nc.gpsimd.partition_all_reduce(gm[:], pm[:], channels=P,
                                           reduce_op=bass_isa.ReduceOp.max)
            negm = fix.tile([P, 1], f32)
            nc.vector.tensor_scalar_mul(negm[:], gm[:], -1.0)
            e_sb = fix.tile([P, P], f32)
            s_sb = fix.tile([P, 1], f32)
            nc.scalar.activation(out=e_sb[:], in_=r_sb[:],
                                 func=mybir.ActivationFunctionType.Exp,
                                 bias=negm[:], scale=1.0, accum_out=s_sb[:])
            stot = fix.tile([P, 1], f32)
            nc.gpsimd.partition_all_reduce(stot[:], s_sb[:], channels=P,
                                           reduce_op=bass_isa.ReduceOp.add)
            rec = fix.tile([P, 1], f32)
            nc.vector.reciprocal(rec[:], stot[:])
            alpha_sb = fix.tile([P, P], f32)
            nc.scalar.activation(out=alpha_sb[:], in_=e_sb[:],
                                 func=mybir.ActivationFunctionType.Copy,
                                 scale=rec[:])
            nc.sync.dma_start(out=alpha_d[:], in_=alpha_sb[:])

    nc.compile()
    return nc


def _get_program():
    if "nc" not in _CACHE:
        _CACHE["nc"] = _build_program()
    return _CACHE["nc"]


def make_in_maps(input_tensor, indices, values):
    input_tensor = np.asarray(input_tensor)
    indices = np.asarray(indices)
    values = np.ascontiguousarray(np.asarray(values, dtype=np.float32))
    in_maps = []
    for m in range(NCORES):
        r0, r1 = m * R, (m + 1) * R
        in_maps.append({
            "rows": np.ascontiguousarray(
                input_tensor[r0:r1].astype(np.float16)),
            "idx": np.ascontiguousarray(indices[r0:r1].astype(np.int32)),
            "vals": values[r0:r1],
        })
    return in_maps


def kernel(input_tensor, indices, values, k=K, **_unused):
    assert int(k) == K
    from concourse.bass_utils import run_bass_kernel_spmd

    nc = _get_program()
    in_maps = make_in_maps(input_tensor, indices, values)
    res = run_bass_kernel_spmd(nc, in_maps, list(range(NCORES)))
    out0 = res.results[0]
    alpha = np.asarray(out0["alpha"], dtype=np.float32).reshape(C)
    reduced = np.asarray(out0["red"], dtype=np.float32).reshape(C)
    return alpha, reduced


# revision 22
# speedup vs baseline: 1.1392x; 1.0340x over previous
"""Trainium2 Bass kernel for NeighborAggregator (gather -> segment_sum -> softmax).

Strategy (8 NeuronCores, SPMD):
  - Row-shard the N=16384 instances: each core owns R=2048 rows = 65536 edges.
  - Host converts input_tensor to bf16, halving the streamed bytes: each core
    streams its 64MiB row shard through SBUF in 16 blocks of 128 rows
    (32KiB/partition); gathers d[p,j] = row_p[idx[p,j]] on-chip with one
    gpsimd ap_gather per block (bf16 row fits the 32KB/partition limit in a
    single call) + diagonal-mask select + strided reduce.
  - contrib = d * values (bf16 for the PE masks).
  - Segment-sum over C=16384 columns via PE outer-product histogram in bf16:
    tile t = one edge per partition,
      a_m[p, h] = contrib_p * [hi_p == h]   (hi = idx >> 7)
      b_eq[p, l] = [lo_p == l]              (lo = idx & 127)
    a-side built on DVE with batched broadcast-iota tensor_tensor ops
    (TB=8 tiles per instruction); b-side split between DVE (batched TT) and
    the scalar engine (Relu(1 - (iota - lo)^2) = one-hot, Square+Relu with
    per-partition bias) to balance engine load. PE accumulates
    psum[h,l] += a_m^T @ b_eq over all 512 tiles at bf16 rate.
  - AllReduce the [128,128] f32 partials across the 8 cores, then softmax
    on-device; every core writes the identical full outputs.
"""

import sys

for _p in ("/opt/trn_rl_repo", "/root/.axon_site/_ro/trn_rl_repo"):
    if _p not in sys.path:
        sys.path.append(_p)

import numpy as np

N = 16384
C = 16384
K = 32
NCORES = 8
P = 128

R = N // NCORES          # rows per core
NBLK = R // P            # row blocks per core (16)
EPP = NBLK * K           # edges per partition (512) == number of tiles
TB = 8                   # tiles per batched mask group
NGRP = EPP // TB         # mask groups (64)

_CACHE = {}


def _build_program():
    from concourse import bacc, bass_isa, mybir, tile

    f32 = mybir.dt.float32
    bf16 = mybir.dt.bfloat16
    fp16 = mybir.dt.float16
    i32 = mybir.dt.int32
    i16 = mybir.dt.int16

    nc = bacc.Bacc("TRN2", target_bir_lowering=False, debug=False,
                   num_devices=NCORES)

    rows_d = nc.dram_tensor("rows", [R, C], fp16, kind="ExternalInput")
    idx_d = nc.dram_tensor("idx", [R, K], i32, kind="ExternalInput")
    vals_d = nc.dram_tensor("vals", [R, K], f32, kind="ExternalInput")
    alpha_d = nc.dram_tensor("alpha", [P, P], f32, kind="ExternalOutput")
    red_d = nc.dram_tensor("red", [P, P], f32, kind="ExternalOutput")

    with tile.TileContext(nc) as tc:
        with tc.tile_pool(name="fix", bufs=1) as fix, \
             tc.tile_pool(name="rows", bufs=3) as rows_pool, \
             tc.tile_pool(name="work", bufs=3) as work, \
             tc.tile_pool(name="maeq", bufs=3) as maeq_pool, \
             tc.tile_pool(name="mam", bufs=4) as mam_pool, \
             tc.tile_pool(name="mb", bufs=3) as mb_pool, \
             tc.tile_pool(name="mbact", bufs=20) as mbact_pool, \
             tc.tile_pool(name="psum", bufs=1, space="PSUM") as psum_pool, \
             tc.tile_pool(name="dram", bufs=1, space="DRAM") as dram_pool:

            # ---- one-time constants ----
            iota_i = fix.tile([P, P], i32)
            nc.gpsimd.iota(iota_i[:], pattern=[[1, P]], base=0,
                           channel_multiplier=0)
            iota_b = fix.tile([P, P], f32)
            nc.vector.tensor_copy(out=iota_b[:], in_=iota_i[:])

            # diagonal mask for ap_gather extraction:
            # dmask[p, s*16 + k] = (k == p % 16)
            fmod = fix.tile([P, K * 16], i32)
            nc.gpsimd.iota(fmod[:], pattern=[[0, K], [1, 16]], base=0,
                           channel_multiplier=0)
            pidx = fix.tile([P, 1], i32)
            nc.gpsimd.iota(pidx[:], pattern=[[0, 1]], base=0,
                           channel_multiplier=1)
            pmod = fix.tile([P, 1], i32)
            nc.vector.tensor_scalar(out=pmod[:], in0=pidx[:], scalar1=15,
                                    scalar2=None,
                                    op0=mybir.AluOpType.bitwise_and)
            dmask = fix.tile([P, K * 16], fp16)
            nc.vector.tensor_tensor(out=dmask[:], in0=fmod[:],
                                    in1=pmod[:].to_broadcast([P, K * 16]),
                                    op=mybir.AluOpType.is_equal)

            # ---- load idx/vals: [R, K] -> [P, NBLK*K] ----
            idx_sb = fix.tile([P, EPP], i32)
            nc.sync.dma_start(
                out=idx_sb[:].rearrange("p (b k) -> p b k", k=K),
                in_=idx_d[:].rearrange("(b p) k -> p b k", p=P))
            vals_sb = fix.tile([P, EPP], f32)
            nc.sync.dma_start(
                out=vals_sb[:].rearrange("p (b k) -> p b k", k=K),
                in_=vals_d[:].rearrange("(b p) k -> p b k", p=P))

            # ---- idx-derived streams ----
            # pair index for the f32-pair gather from bf16 rows
            idxp_i = fix.tile([P, EPP], i32)
            nc.vector.tensor_scalar(out=idxp_i[:], in0=idx_sb[:], scalar1=1,
                                    scalar2=None,
                                    op0=mybir.AluOpType.logical_shift_right)
            idx16 = fix.tile([P, EPP], i16)
            nc.vector.tensor_copy(out=idx16[:], in_=idxp_i[:])
            # half-bit hb = idx & 1, split vals into v0/v1 = vals*(1-hb), vals*hb
            hb_i = fix.tile([P, EPP], i32)
            nc.vector.tensor_scalar(out=hb_i[:], in0=idx_sb[:], scalar1=1,
                                    scalar2=None,
                                    op0=mybir.AluOpType.bitwise_and)
            hbf = fix.tile([P, EPP], f32)
            nc.vector.tensor_copy(out=hbf[:], in_=hb_i[:])
            v1 = fix.tile([P, EPP], f32)
            nc.vector.tensor_tensor(out=v1[:], in0=vals_sb[:], in1=hbf[:],
                                    op=mybir.AluOpType.mult)
            v0 = fix.tile([P, EPP], f32)
            nc.vector.tensor_tensor(out=v0[:], in0=vals_sb[:], in1=v1[:],
                                    op=mybir.AluOpType.subtract)
            hi_i = fix.tile([P, EPP], i32)
            nc.vector.tensor_scalar(out=hi_i[:], in0=idx_sb[:], scalar1=7,
                                    scalar2=None,
                                    op0=mybir.AluOpType.logical_shift_right)
            hif = fix.tile([P, EPP], f32)
            nc.vector.tensor_copy(out=hif[:], in_=hi_i[:])
            lo_i = fix.tile([P, EPP], i32)
            nc.vector.tensor_scalar(out=lo_i[:], in0=idx_sb[:], scalar1=127,
                                    scalar2=None,
                                    op0=mybir.AluOpType.bitwise_and)
            lof = fix.tile([P, EPP], f32)
            nc.vector.tensor_copy(out=lof[:], in_=lo_i[:])
            # negated f32 lo for the scalar-engine one-hot bias
            nlof = fix.tile([P, EPP], f32)
            nc.vector.tensor_scalar(out=nlof[:], in0=lo_i[:], scalar1=-1,
                                    scalar2=None, op0=mybir.AluOpType.mult)

            cf32 = fix.tile([P, EPP], f32)

            # two independent accumulators: even groups (DVE-built masks)
            # go to psumA inline; odd groups (scalar-engine lo one-hots)
            # are deferred to the end of each block into psumB, so the
            # in-order PE queue never blocks mid-block on the slower
            # scalar engine.
            psumA = psum_pool.tile([P, P], f32)
            psumB = psum_pool.tile([P, P], f32)
            n_tiles_per_bank = EPP // 2
            mm_count = {"A": 0, "B": 0}
            pending_b = []

            def mm(bank, psum_t, lhsT, rhs):
                i = mm_count[bank]
                nc.tensor.matmul(out=psum_t[:], lhsT=lhsT, rhs=rhs,
                                 start=(i == 0),
                                 stop=(i == n_tiles_per_bank - 1),
                                 skip_group_check=True)
                mm_count[bank] = i + 1

            def flush_pending():
                for lhsT, rhs in pending_b:
                    mm("B", psumB, lhsT, rhs)
                pending_b.clear()

            def emit_select(rb):
                # gather+select for block rb: d values -> cf32[:, block slice]
                r0 = rb * P
                ks = slice(rb * K, (rb + 1) * K)
                rows_blk = rows_pool.tile([P, C // 2], f32, tag="rows")
                nc.sync.dma_start(out=rows_blk[:],
                                  in_=rows_d[r0:r0 + P, :].bitcast(f32))

                # on-chip gather of the f32 pair holding each target fp16
                # (idx i applied to all 16 partitions of each Q7 group):
                # gath[p, s*16+k] = rows_blk[p, idxp[(g*16)+k, s]]
                gath = work.tile([P, K * 16], f32, tag="gath")
                nc.gpsimd.ap_gather(out_ap=gath[:], in_ap=rows_blk[:],
                                    idxs_ap=idx16[:, ks],
                                    channels=P, num_elems=C // 2, d=1,
                                    num_idxs=K * 16)
                gb = gath[:].bitcast(fp16).rearrange("p (x h) -> p x h", h=2)
                sel0 = work.tile([P, K * 16], fp16, tag="sel0")
                nc.vector.tensor_tensor(out=sel0[:], in0=gb[:, :, 0],
                                        in1=dmask[:],
                                        op=mybir.AluOpType.mult)
                sel1 = work.tile([P, K * 16], fp16, tag="sel1")
                nc.vector.tensor_tensor(out=sel1[:], in0=gb[:, :, 1],
                                        in1=dmask[:],
                                        op=mybir.AluOpType.mult)
                dv0 = work.tile([P, K], f32, tag="dv0")
                nc.vector.tensor_reduce(
                    out=dv0[:],
                    in_=sel0[:].rearrange("p (s k) -> p s k", k=16),
                    axis=mybir.AxisListType.X,
                    op=mybir.AluOpType.add)
                dv1 = work.tile([P, K], f32, tag="dv1")
                nc.vector.tensor_reduce(
                    out=dv1[:],
                    in_=sel1[:].rearrange("p (s k) -> p s k", k=16),
                    axis=mybir.AxisListType.X,
                    op=mybir.AluOpType.add)
                # contrib = dv0*v0 + dv1*v1 (selects the right fp16 half)
                c0t = work.tile([P, K], f32, tag="c0t")
                nc.vector.tensor_tensor(out=c0t[:], in0=dv0[:],
                                        in1=v0[:, ks],
                                        op=mybir.AluOpType.mult)
                c1t = work.tile([P, K], f32, tag="c1t")
                nc.vector.tensor_tensor(out=c1t[:], in0=dv1[:],
                                        in1=v1[:, ks],
                                        op=mybir.AluOpType.mult)
                nc.vector.tensor_tensor(out=cf32[:, ks], in0=c0t[:],
                                        in1=c1t[:], op=mybir.AluOpType.add)

            def emit_masks(rb):
                # mask build + matmuls for block rb (runs one block behind
                # the gather/select so cross-engine waits resolve early)
                for gg in range(K // TB):
                    g = rb * (K // TB) + gg
                    t0 = g * TB

                    hi_bc = hif[:, t0:t0 + TB][:, :, None].broadcast_to(
                        [P, TB, P])
                    io_bc = iota_b[:][:, None, :].broadcast_to([P, TB, P])
                    c_bc = cf32[:, t0:t0 + TB][:, :, None].broadcast_to(
                        [P, TB, P])

                    a_eq = maeq_pool.tile([P, TB, P], f32, tag="a_eq")
                    nc.vector.tensor_tensor(out=a_eq[:], in0=hi_bc,
                                            in1=io_bc,
                                            op=mybir.AluOpType.is_equal)
                    a_m = mam_pool.tile([P, TB, P], f32, tag="a_m")
                    nc.vector.tensor_tensor(out=a_m[:], in0=a_eq[:],
                                            in1=c_bc,
                                            op=mybir.AluOpType.mult)

                    if g % 2 == 1:
                        # scalar engine builds the lo one-hot via
                        # Relu(1 - (iota - lo)^2); matmuls deferred
                        for t in range(TB):
                            T = t0 + t
                            sq = mbact_pool.tile([P, P], f32, tag="sq")
                            nc.scalar.activation(
                                out=sq[:], in_=iota_b[:],
                                func=mybir.ActivationFunctionType.Square,
                                bias=nlof[:, T:T + 1], scale=1.0)
                            bt = mbact_pool.tile([P, P], f32, tag="bt")
                            nc.scalar.activation(
                                out=bt[:], in_=sq[:],
                                func=mybir.ActivationFunctionType.Relu,
                                bias=1.0, scale=-1.0)
                            pending_b.append((a_m[:, t, :], bt[:]))
                    else:
                        lo_bc = lof[:, t0:t0 + TB][:, :, None].broadcast_to(
                            [P, TB, P])
                        b_eq = mb_pool.tile([P, TB, P], f32, tag="b_eq")
                        nc.vector.tensor_tensor(out=b_eq[:], in0=lo_bc,
                                                in1=io_bc,
                                                op=mybir.AluOpType.is_equal)
                        for t in range(TB):
                            mm("A", psumA, a_m[:, t, :], b_eq[:, t, :])

                flush_pending()

            for rb in range(NBLK):
                emit_select(rb)
                emit_masks(rb)

            assert mm_count["A"] == mm_count["B"] == n_tiles_per_bank

            red_tmp = fix.tile([P, P], f32)
            nc.vector.tensor_copy(out=red_tmp[:], in_=psumA[:])
            red_sb = fix.tile([P, P], f32)
            nc.vector.tensor_tensor(out=red_sb[:], in0=red_tmp[:],
                                    in1=psumB[:], op=mybir.AluOpType.add)

            partial = dram_pool.tile([P, P], f32)
            nc.sync.dma_start(out=partial[:], in_=red_sb[:])
            allred = dram_pool.tile([P, P], f32)
            nc.gpsimd.collective_compute(
                "AllReduce",
                mybir.AluOpType.add,
                replica_groups=[list(range(NCORES))],
                ins=[partial[:].opt()],
                outs=[allred[:].opt()],
            )
            r_sb = fix.tile([P, P], f32)
            nc.sync.dma_start(out=r_sb[:], in_=allred[:])
            nc.sync.dma_start(out=red_d[:], in_=r_sb[:])

            # softmax over all 16384 entries of r_sb
            pm = fix.tile([P, 1], f32)
            nc.vector.tensor_reduce(out=pm[:], in_=r_sb[:],
                                    axis=mybir.AxisListType.X,
                                    op=mybir.AluOpType.max)
            gm = fix.tile([P, 1], f32)
            nc.gpsimd.partition_all_reduce(gm[:], pm[:], channels=P,
                                           reduce_op=bass_isa.ReduceOp.max)
            negm = fix.tile([P, 1], f32)
            nc.vector.tensor_scalar_mul(negm[:], gm[:], -1.0)
            e_sb = fix.tile([P, P], f32)
            s_sb = fix.tile([P, 1], f32)
            nc.scalar.activation(out=e_sb[:], in_=r_sb[:],
                                 func=mybir.ActivationFunctionType.Exp,
                                 bias=negm[:], scale=1.0, accum_out=s_sb[:])
            stot = fix.tile([P, 1], f32)
            nc.gpsimd.partition_all_reduce(stot[:], s_sb[:], channels=P,
                                           reduce_op=bass_isa.ReduceOp.add)
            rec = fix.tile([P, 1], f32)
            nc.vector.reciprocal(rec[:], stot[:])
            alpha_sb = fix.tile([P, P], f32)
            nc.scalar.activation(out=alpha_sb[:], in_=e_sb[:],
                                 func=mybir.ActivationFunctionType.Copy,
                                 scale=rec[:])
            nc.sync.dma_start(out=alpha_d[:], in_=alpha_sb[:])

    nc.compile()
    return nc


def _get_program():
    if "nc" not in _CACHE:
        _CACHE["nc"] = _build_program()
    return _CACHE["nc"]


def make_in_maps(input_tensor, indices, values):
    input_tensor = np.asarray(input_tensor)
    indices = np.asarray(indices)
    values = np.ascontiguousarray(np.asarray(values, dtype=np.float32))
    in_maps = []
    for m in range(NCORES):
        r0, r1 = m * R, (m + 1) * R
        in_maps.append({
            "rows": np.ascontiguousarray(
                input_tensor[r0:r1].astype(np.float16)),
            "idx": np.ascontiguousarray(indices[r0:r1].astype(np.int32)),
            "vals": values[r0:r1],
        })
    return in_maps


def kernel(input_tensor, indices, values, k=K, **_unused):
    assert int(k) == K
    from concourse.bass_utils import run_bass_kernel_spmd

    nc = _get_program()
    in_maps = make_in_maps(input_tensor, indices, values)
    res = run_bass_kernel_spmd(nc, in_maps, list(range(NCORES)))
    out0 = res.results[0]
    alpha = np.asarray(out0["alpha"], dtype=np.float32).reshape(C)
    reduced = np.asarray(out0["red"], dtype=np.float32).reshape(C)
    return alpha, reduced


# revision 25
# speedup vs baseline: 1.1567x; 1.0153x over previous
"""Trainium2 Bass kernel for NeighborAggregator (gather -> segment_sum -> softmax).

Strategy (8 NeuronCores, SPMD):
  - Row-shard the N=16384 instances: each core owns R=2048 rows = 65536 edges.
  - Host converts input_tensor to fp16 (d quantization ~1e-4, well inside the
    2e-2 gate), halving the streamed bytes: each core streams its 64MiB row
    shard through SBUF in 16 blocks of 128 rows (32KiB/partition), viewed as
    f32 pairs; gathers the f32 pair holding each d[p,j] = row_p[idx[p,j]]
    on-chip with one gpsimd ap_gather per block, then selects the fp16 half
    with a diagonal-mask multiply + strided reduce + the v0/v1 = vals*(1-hb),
    vals*hb split (hb = idx & 1), yielding contrib = d * values in f32.
  - Segment-sum over C=16384 columns via PE outer-product histogram in f32
    (f32 masks keep contrib exact; PE has headroom): tile t = one edge per
    partition,
      a_m[p, h] = contrib_p * [hi_p == h]   (hi = idx >> 7)
      b_eq[p, l] = [lo_p == l]              (lo = idx & 127)
    a-side built on DVE with batched broadcast-iota tensor_tensor ops
    (TB=8 tiles per instruction); b-side alternates per group between DVE
    (batched TT) and the scalar engine (Relu(1 - (iota - lo)^2) = one-hot,
    Square+Relu with per-partition bias) to balance engine load. Two PSUM
    banks: DVE-flavor matmuls accumulate inline into psumA, scalar-engine
    flavor matmuls are deferred to each block's end into psumB so the
    in-order PE queue never blocks mid-block on the scalar engine.
  - AllReduce the [128,128] f32 partials (psumA+psumB) across the 8 cores,
    then softmax on-device; every core writes the identical full outputs.
"""

import sys

for _p in ("/opt/trn_rl_repo", "/root/.axon_site/_ro/trn_rl_repo"):
    if _p not in sys.path:
        sys.path.append(_p)

import numpy as np

N = 16384
C = 16384
K = 32
NCORES = 8
P = 128

R = N // NCORES          # rows per core
NBLK = R // P            # row blocks per core (16)
EPP = NBLK * K           # edges per partition (512) == number of tiles
TB = 8                   # tiles per batched mask group
NGRP = EPP // TB         # mask groups (64)

_CACHE = {}


def _build_program():
    from concourse import bacc, bass_isa, mybir, tile

    f32 = mybir.dt.float32
    bf16 = mybir.dt.bfloat16
    fp16 = mybir.dt.float16
    i32 = mybir.dt.int32
    i16 = mybir.dt.int16

    nc = bacc.Bacc("TRN2", target_bir_lowering=False, debug=False,
                   num_devices=NCORES)

    rows_d = nc.dram_tensor("rows", [R, C], fp16, kind="ExternalInput")
    idx_d = nc.dram_tensor("idx", [R, K], i32, kind="ExternalInput")
    vals_d = nc.dram_tensor("vals", [R, K], f32, kind="ExternalInput")
    alpha_d = nc.dram_tensor("alpha", [P, P], f32, kind="ExternalOutput")
    red_d = nc.dram_tensor("red", [P, P], f32, kind="ExternalOutput")

    with tile.TileContext(nc) as tc:
        with tc.tile_pool(name="fix", bufs=1) as fix, \
             tc.tile_pool(name="rows", bufs=3) as rows_pool, \
             tc.tile_pool(name="work", bufs=3) as work, \
             tc.tile_pool(name="maeq", bufs=3) as maeq_pool, \
             tc.tile_pool(name="mam", bufs=4) as mam_pool, \
             tc.tile_pool(name="mb", bufs=3) as mb_pool, \
             tc.tile_pool(name="mbact", bufs=20) as mbact_pool, \
             tc.tile_pool(name="psum", bufs=1, space="PSUM") as psum_pool, \
             tc.tile_pool(name="dram", bufs=1, space="DRAM") as dram_pool:

            # ---- one-time constants ----
            iota_i = fix.tile([P, P], i32)
            nc.gpsimd.iota(iota_i[:], pattern=[[1, P]], base=0,
                           channel_multiplier=0)
            iota_b = fix.tile([P, P], f32)
            nc.vector.tensor_copy(out=iota_b[:], in_=iota_i[:])

            # diagonal mask for ap_gather extraction:
            # dmask[p, s*16 + k] = (k == p % 16)
            fmod = fix.tile([P, K * 16], i32)
            nc.gpsimd.iota(fmod[:], pattern=[[0, K], [1, 16]], base=0,
                           channel_multiplier=0)
            pidx = fix.tile([P, 1], i32)
            nc.gpsimd.iota(pidx[:], pattern=[[0, 1]], base=0,
                           channel_multiplier=1)
            pmod = fix.tile([P, 1], i32)
            nc.vector.tensor_scalar(out=pmod[:], in0=pidx[:], scalar1=15,
                                    scalar2=None,
                                    op0=mybir.AluOpType.bitwise_and)

            # dummy ap_gather so the Q7 gather library loads during warmup
            # (otherwise the ~20us load sits behind block 0's rows DMA)
            warm_idx = fix.tile([P, 2], i16)
            nc.gpsimd.memset(warm_idx[:], 0)
            warm_out = fix.tile([P, 32], i32)
            nc.gpsimd.ap_gather(out_ap=warm_out[:], in_ap=iota_i[:],
                                idxs_ap=warm_idx[:], channels=P,
                                num_elems=P, d=1, num_idxs=32)
            dmask = fix.tile([P, K * 16], fp16)
            nc.vector.tensor_tensor(out=dmask[:], in0=fmod[:],
                                    in1=pmod[:].to_broadcast([P, K * 16]),
                                    op=mybir.AluOpType.is_equal)

            # ---- load idx/vals: [R, K] -> [P, NBLK*K] ----
            idx_sb = fix.tile([P, EPP], i32)
            nc.sync.dma_start(
                out=idx_sb[:].rearrange("p (b k) -> p b k", k=K),
                in_=idx_d[:].rearrange("(b p) k -> p b k", p=P))
            # issue on the scalar queue so it overlaps the idx DMA (both are
            # slow 2048x128B strided transfers; serial on sync costs ~15us)
            vals_sb = fix.tile([P, EPP], f32)
            nc.scalar.dma_start(
                out=vals_sb[:].rearrange("p (b k) -> p b k", k=K),
                in_=vals_d[:].rearrange("(b p) k -> p b k", p=P))

            # ---- idx-derived streams ----
            # pair index for the f32-pair gather from bf16 rows
            idxp_i = fix.tile([P, EPP], i32)
            nc.vector.tensor_scalar(out=idxp_i[:], in0=idx_sb[:], scalar1=1,
                                    scalar2=None,
                                    op0=mybir.AluOpType.logical_shift_right)
            idx16 = fix.tile([P, EPP], i16)
            nc.vector.tensor_copy(out=idx16[:], in_=idxp_i[:])
            # half-bit hb = idx & 1, split vals into v0/v1 = vals*(1-hb), vals*hb
            hb_i = fix.tile([P, EPP], i32)
            nc.vector.tensor_scalar(out=hb_i[:], in0=idx_sb[:], scalar1=1,
                                    scalar2=None,
                                    op0=mybir.AluOpType.bitwise_and)
            hbf = fix.tile([P, EPP], f32)
            nc.vector.tensor_copy(out=hbf[:], in_=hb_i[:])
            v1 = fix.tile([P, EPP], f32)
            nc.vector.tensor_tensor(out=v1[:], in0=vals_sb[:], in1=hbf[:],
                                    op=mybir.AluOpType.mult)
            v0 = fix.tile([P, EPP], f32)
            nc.vector.tensor_tensor(out=v0[:], in0=vals_sb[:], in1=v1[:],
                                    op=mybir.AluOpType.subtract)
            hi_i = fix.tile([P, EPP], i32)
            nc.vector.tensor_scalar(out=hi_i[:], in0=idx_sb[:], scalar1=7,
                                    scalar2=None,
                                    op0=mybir.AluOpType.logical_shift_right)
            hif = fix.tile([P, EPP], f32)
            nc.vector.tensor_copy(out=hif[:], in_=hi_i[:])
            lo_i = fix.tile([P, EPP], i32)
            nc.vector.tensor_scalar(out=lo_i[:], in0=idx_sb[:], scalar1=127,
                                    scalar2=None,
                                    op0=mybir.AluOpType.bitwise_and)
            lof = fix.tile([P, EPP], f32)
            nc.vector.tensor_copy(out=lof[:], in_=lo_i[:])
            # negated f32 lo for the scalar-engine one-hot bias
            nlof = fix.tile([P, EPP], f32)
            nc.vector.tensor_scalar(out=nlof[:], in0=lo_i[:], scalar1=-1,
                                    scalar2=None, op0=mybir.AluOpType.mult)

            cf32 = fix.tile([P, EPP], f32)

            # two independent accumulators: even groups (DVE-built masks)
            # go to psumA inline; odd groups (scalar-engine lo one-hots)
            # are deferred to the end of each block into psumB, so the
            # in-order PE queue never blocks mid-block on the slower
            # scalar engine.
            psumA = psum_pool.tile([P, P], f32)
            psumB = psum_pool.tile([P, P], f32)
            n_tiles_per_bank = EPP // 2
            mm_count = {"A": 0, "B": 0}
            pending_b = []

            def mm(bank, psum_t, lhsT, rhs):
                i = mm_count[bank]
                nc.tensor.matmul(out=psum_t[:], lhsT=lhsT, rhs=rhs,
                                 start=(i == 0),
                                 stop=(i == n_tiles_per_bank - 1),
                                 skip_group_check=True)
                mm_count[bank] = i + 1

            def flush_pending():
                for lhsT, rhs in pending_b:
                    mm("B", psumB, lhsT, rhs)
                pending_b.clear()

            def emit_select(rb):
                # gather+select for block rb: d values -> cf32[:, block slice]
                r0 = rb * P
                ks = slice(rb * K, (rb + 1) * K)
                rows_blk = rows_pool.tile([P, C // 2], f32, tag="rows")
                nc.sync.dma_start(out=rows_blk[:],
                                  in_=rows_d[r0:r0 + P, :].bitcast(f32))

                # on-chip gather of the f32 pair holding each target fp16
                # (idx i applied to all 16 partitions of each Q7 group):
                # gath[p, s*16+k] = rows_blk[p, idxp[(g*16)+k, s]]
                gath = work.tile([P, K * 16], f32, tag="gath")
                nc.gpsimd.ap_gather(out_ap=gath[:], in_ap=rows_blk[:],
                                    idxs_ap=idx16[:, ks],
                                    channels=P, num_elems=C // 2, d=1,
                                    num_idxs=K * 16)
                gb = gath[:].bitcast(fp16).rearrange("p (x h) -> p x h", h=2)
                sel0 = work.tile([P, K * 16], fp16, tag="sel0")
                nc.vector.tensor_tensor(out=sel0[:], in0=gb[:, :, 0],
                                        in1=dmask[:],
                                        op=mybir.AluOpType.mult)
                sel1 = work.tile([P, K * 16], fp16, tag="sel1")
                nc.vector.tensor_tensor(out=sel1[:], in0=gb[:, :, 1],
                                        in1=dmask[:],
                                        op=mybir.AluOpType.mult)
                dv0 = work.tile([P, K], f32, tag="dv0")
                nc.vector.tensor_reduce(
                    out=dv0[:],
                    in_=sel0[:].rearrange("p (s k) -> p s k", k=16),
                    axis=mybir.AxisListType.X,
                    op=mybir.AluOpType.add)
                dv1 = work.tile([P, K], f32, tag="dv1")
                nc.vector.tensor_reduce(
                    out=dv1[:],
                    in_=sel1[:].rearrange("p (s k) -> p s k", k=16),
                    axis=mybir.AxisListType.X,
                    op=mybir.AluOpType.add)
                # contrib = dv0*v0 + dv1*v1 (selects the right fp16 half)
                c0t = work.tile([P, K], f32, tag="c0t")
                nc.vector.tensor_tensor(out=c0t[:], in0=dv0[:],
                                        in1=v0[:, ks],
                                        op=mybir.AluOpType.mult)
                c1t = work.tile([P, K], f32, tag="c1t")
                nc.vector.tensor_tensor(out=c1t[:], in0=dv1[:],
                                        in1=v1[:, ks],
                                        op=mybir.AluOpType.mult)
                nc.vector.tensor_tensor(out=cf32[:, ks], in0=c0t[:],
                                        in1=c1t[:], op=mybir.AluOpType.add)

            def emit_masks(rb):
                # mask build + matmuls for block rb (runs one block behind
                # the gather/select so cross-engine waits resolve early)
                for gg in range(K // TB):
                    g = rb * (K // TB) + gg
                    t0 = g * TB

                    hi_bc = hif[:, t0:t0 + TB][:, :, None].broadcast_to(
                        [P, TB, P])
                    io_bc = iota_b[:][:, None, :].broadcast_to([P, TB, P])
                    c_bc = cf32[:, t0:t0 + TB][:, :, None].broadcast_to(
                        [P, TB, P])

                    a_eq = maeq_pool.tile([P, TB, P], f32, tag="a_eq")
                    nc.vector.tensor_tensor(out=a_eq[:], in0=hi_bc,
                                            in1=io_bc,
                                            op=mybir.AluOpType.is_equal)
                    a_m = mam_pool.tile([P, TB, P], f32, tag="a_m")
                    nc.vector.tensor_tensor(out=a_m[:], in0=a_eq[:],
                                            in1=c_bc,
                                            op=mybir.AluOpType.mult)

                    if g % 2 == 1:
                        # scalar engine builds the lo one-hot via
                        # Relu(1 - (iota - lo)^2); matmuls deferred
                        for t in range(TB):
                            T = t0 + t
                            sq = mbact_pool.tile([P, P], f32, tag="sq")
                            nc.scalar.activation(
                                out=sq[:], in_=iota_b[:],
                                func=mybir.ActivationFunctionType.Square,
                                bias=nlof[:, T:T + 1], scale=1.0)
                            bt = mbact_pool.tile([P, P], f32, tag="bt")
                            nc.scalar.activation(
                                out=bt[:], in_=sq[:],
                                func=mybir.ActivationFunctionType.Relu,
                                bias=1.0, scale=-1.0)
                            pending_b.append((a_m[:, t, :], bt[:]))
                    else:
                        lo_bc = lof[:, t0:t0 + TB][:, :, None].broadcast_to(
                            [P, TB, P])
                        b_eq = mb_pool.tile([P, TB, P], f32, tag="b_eq")
                        nc.vector.tensor_tensor(out=b_eq[:], in0=lo_bc,
                                                in1=io_bc,
                                                op=mybir.AluOpType.is_equal)
                        for t in range(TB):
                            mm("A", psumA, a_m[:, t, :], b_eq[:, t, :])

                flush_pending()

            for rb in range(NBLK):
                emit_select(rb)
                emit_masks(rb)

            assert mm_count["A"] == mm_count["B"] == n_tiles_per_bank

            red_tmp = fix.tile([P, P], f32)
            nc.vector.tensor_copy(out=red_tmp[:], in_=psumA[:])
            red_sb = fix.tile([P, P], f32)
            nc.vector.tensor_tensor(out=red_sb[:], in0=red_tmp[:],
                                    in1=psumB[:], op=mybir.AluOpType.add)

            partial = dram_pool.tile([P, P], f32)
            nc.sync.dma_start(out=partial[:], in_=red_sb[:])
            allred = dram_pool.tile([P, P], f32)
            nc.gpsimd.collective_compute(
                "AllReduce",
                mybir.AluOpType.add,
                replica_groups=[list(range(NCORES))],
                ins=[partial[:].opt()],
                outs=[allred[:].opt()],
            )
            r_sb = fix.tile([P, P], f32)
            nc.sync.dma_start(out=r_sb[:], in_=allred[:])
            nc.sync.dma_start(out=red_d[:], in_=r_sb[:])

            # softmax over all 16384 entries of r_sb
            pm = fix.tile([P, 1], f32)
            nc.vector.tensor_reduce(out=pm[:], in_=r_sb[:],
                                    axis=mybir.AxisListType.X,
                                    op=mybir.AluOpType.max)
            gm = fix.tile([P, 1], f32)
            nc.gpsimd.partition_all_reduce(gm[:], pm[:], channels=P,
                                           reduce_op=bass_isa.ReduceOp.max)
            negm = fix.tile([P, 1], f32)
            nc.vector.tensor_scalar_mul(negm[:], gm[:], -1.0)
            e_sb = fix.tile([P, P], f32)
            s_sb = fix.tile([P, 1], f32)
            nc.scalar.activation(out=e_sb[:], in_=r_sb[:],
                                 func=mybir.ActivationFunctionType.Exp,
                                 bias=negm[:], scale=1.0, accum_out=s_sb[:])
            stot = fix.tile([P, 1], f32)
            nc.gpsimd.partition_all_reduce(stot[:], s_sb[:], channels=P,
                                           reduce_op=bass_isa.ReduceOp.add)
            rec = fix.tile([P, 1], f32)
            nc.vector.reciprocal(rec[:], stot[:])
            alpha_sb = fix.tile([P, P], f32)
            nc.scalar.activation(out=alpha_sb[:], in_=e_sb[:],
                                 func=mybir.ActivationFunctionType.Copy,
                                 scale=rec[:])
            nc.sync.dma_start(out=alpha_d[:], in_=alpha_sb[:])

    nc.compile()
    return nc


def _get_program():
    if "nc" not in _CACHE:
        _CACHE["nc"] = _build_program()
    return _CACHE["nc"]


def make_in_maps(input_tensor, indices, values):
    input_tensor = np.asarray(input_tensor)
    indices = np.asarray(indices)
    values = np.ascontiguousarray(np.asarray(values, dtype=np.float32))
    in_maps = []
    for m in range(NCORES):
        r0, r1 = m * R, (m + 1) * R
        in_maps.append({
            "rows": np.ascontiguousarray(
                input_tensor[r0:r1].astype(np.float16)),
            "idx": np.ascontiguousarray(indices[r0:r1].astype(np.int32)),
            "vals": values[r0:r1],
        })
    return in_maps


def kernel(input_tensor, indices, values, k=K, **_unused):
    assert int(k) == K
    from concourse.bass_utils import run_bass_kernel_spmd

    nc = _get_program()
    in_maps = make_in_maps(input_tensor, indices, values)
    res = run_bass_kernel_spmd(nc, in_maps, list(range(NCORES)))
    out0 = res.results[0]
    alpha = np.asarray(out0["alpha"], dtype=np.float32).reshape(C)
    reduced = np.asarray(out0["red"], dtype=np.float32).reshape(C)
    return alpha, reduced
